# revision 10
# baseline (speedup 1.0000x reference)
"""Trainium2 Bass kernel for the MoE layer (4 routed top-2 + 2 shared experts).

Strategy (v2): exact expert-parallel routing on the host + fp8 DoubleRow
matmuls with 3-term residual compensation on the device.

  - Host computes the fp32 gating softmax/top-2 and builds a global work
    list of (expert, token) pairs: 2 shared experts x all 8192 tokens +
    top-2 routed picks (16384 pairs) = 32768 pairs total. The list is cut
    into 24 slots of TPH=1408 tokens (one expert per slot, zero-coef
    padding) and distributed 3 slots (phases) per core: 4224 pairs/core.
  - Each phase streams one expert's weights and applies the expert MLP to
    its tokens; per-token fp32 coefficients (0.5 for shared, softmax prob
    for routed) are applied at the output drain; host scatter-adds slot
    outputs back into the full [8192, 1024] output.
  - All matmuls run as fp8e4m3 DoubleRow (0.5 cycles/row, 256-deep
    contraction) with 3-term residual compensation, which costs 0.75x of a
    bf16 schedule but keeps ~bf16 accuracy:
        x@W ~= x8@W8 + rx8@W8 + x8@RW8
    where x8=fp8(x), rx8=fp8(x-x8), W8=fp8(S*W), RW8=fp8(S*W-W8). The
    residuals need no extra scaling (fp8 subnormals cover their range);
    the 1/S weight scale folds into the activation/drain scales.
  - L1 produces h in bf16 (ACT relu) plus h8=fp8 (second ACT relu) and
    rh8=hb-h8 (Pool-engine subtract); L2 consumes (h8, rh8) the same way.
  - L2 is F-blocked (4 blocks of 1024); each block's partial output drains
    through DVE (coef/S2 per-token scale) to bf16 and is summed on host.
"""

import sys

sys.path.insert(0, '/opt/trn_rl_repo')

import numpy as np
import ml_dtypes

import concourse.bass as bass
import concourse.mybir as mybir
import concourse.tile as tile
from concourse import bacc
from concourse.bass_utils import run_bass_kernel_spmd

F8 = ml_dtypes.float8_e4m3
BF16 = ml_dtypes.bfloat16

NCORES = 8
B, S, D, F, O = 4, 2048, 1024, 4096, 1024
E, NS = 4, 2
NEXP = NS + E
TOK = B * S
P = 128
DCH = D // P          # 8 contraction chunks in L1
FCH = F // P          # 32
NFB = 4               # L2 F-blocks
FBC = FCH // NFB      # 8 f-chunks per block
FBW = FBC * P         # 1024 f-cols per block
NPH = 3               # phases (slots) per core
S1, S2 = 32.0, 64.0   # weight pre-scales (powers of 2)

_CACHED = {}
_LAST_TPH = [(1408, False)]


def _ntiles(tph):
    out, n0 = [], 0
    while n0 < tph:
        nl = min(512, tph - n0)
        out.append((n0, nl))
        n0 += nl
    return out


def _build(tph, with_b2=True):
    f32 = mybir.dt.float32
    bf = mybir.dt.bfloat16
    fp8 = mybir.dt.float8e4
    AF = mybir.ActivationFunctionType
    ALU = mybir.AluOpType
    PM = mybir.MatmulPerfMode.DoubleRow
    MT = tph // P
    NT = _ntiles(tph)
    NOH = O // 512

    nc = bacc.Bacc("TRN2", target_bir_lowering=False, debug=False)

    x8_d = nc.dram_tensor("x8", [NPH, P, DCH, tph], fp8, kind="ExternalInput")
    rx8_d = nc.dram_tensor("rx8", [NPH, P, DCH, tph], fp8, kind="ExternalInput")
    w1_d = nc.dram_tensor("w1", [NPH, NFB, P, DCH, FBW], fp8, kind="ExternalInput")
    rw1_d = nc.dram_tensor("rw1", [NPH, NFB, P, DCH, FBW], fp8, kind="ExternalInput")
    w2_d = nc.dram_tensor("w2", [NPH, NFB, P, FBC, O], fp8, kind="ExternalInput")
    rw2_d = nc.dram_tensor("rw2", [NPH, NFB, P, FBC, O], fp8, kind="ExternalInput")
    b1_d = nc.dram_tensor("b1", [NPH, P, FCH], f32, kind="ExternalInput")
    b2_d = nc.dram_tensor("b2", [NPH, 1, O], bf, kind="ExternalInput")
    cf_d = nc.dram_tensor("cf", [NPH, P, MT], f32, kind="ExternalInput")
    out_d = nc.dram_tensor("out", [NPH, NFB, MT, P, O], bf, kind="ExternalOutput")

    with tile.TileContext(nc) as tc:
        with (
            tc.tile_pool(name="xp", bufs=2) as xp,
            tc.tile_pool(name="wp1", bufs=2) as wp1,
            tc.tile_pool(name="wp2", bufs=2) as wp2,
            tc.tile_pool(name="hp", bufs=8) as hp,
            tc.tile_pool(name="hbp", bufs=4) as hbp,
            tc.tile_pool(name="op", bufs=8) as op,
            tc.tile_pool(name="cp", bufs=2) as cp,
            tc.tile_pool(name="consts", bufs=1) as consts,
            tc.tile_pool(name="ps1", bufs=3, space="PSUM") as ps1,
            tc.tile_pool(name="ps2", bufs=4, space="PSUM") as ps2,
        ):
            onesbf = consts.tile([1, P], bf, tag="ones", name="ones")
            nc.vector.memset(onesbf[:], 1.0)

            th = (tph // 2) // P * P  # token-half split for faster first arrival
            for ph in range(NPH):
                x8t = xp.tile([P, DCH, tph], fp8, tag="x8", name=f"x8_{ph}")
                nc.sync.dma_start(x8t[:, :, 0:th], x8_d[ph, :, :, 0:th])
                nc.sync.dma_start(x8t[:, :, th:tph], x8_d[ph, :, :, th:tph])
                rx8t = xp.tile([P, DCH, tph], fp8, tag="rx8", name=f"rx8_{ph}")
                nc.sync.dma_start(rx8t[:, :, 0:th], rx8_d[ph, :, :, 0:th])
                nc.sync.dma_start(rx8t[:, :, th:tph], rx8_d[ph, :, :, th:tph])
                b1t = cp.tile([P, FCH], f32, tag="b1", name=f"b1_{ph}")
                nc.sync.dma_start(b1t[:], b1_d[ph])
                b2t = cp.tile([1, O], bf, tag="b2", name=f"b2_{ph}")
                nc.sync.dma_start(b2t[:], b2_d[ph])
                cft = cp.tile([P, MT], f32, tag="cf", name=f"cf_{ph}")
                nc.sync.dma_start(cft[:], cf_d[ph])

                for fb in range(NFB):
                    fh = FBW // 2
                    w1t = wp1.tile([P, DCH, FBW], fp8, tag="w1", name=f"w1_{ph}_{fb}")
                    nc.sync.dma_start(w1t[:, :, 0:fh], w1_d[ph, fb, :, :, 0:fh])
                    nc.sync.dma_start(w1t[:, :, fh:FBW], w1_d[ph, fb, :, :, fh:FBW])
                    rw1t = wp1.tile([P, DCH, FBW], fp8, tag="rw1", name=f"rw1_{ph}_{fb}")
                    nc.sync.dma_start(rw1t[:, :, 0:fh], rw1_d[ph, fb, :, :, 0:fh])
                    nc.sync.dma_start(rw1t[:, :, fh:FBW], rw1_d[ph, fb, :, :, fh:FBW])
                    w2t = wp2.tile([P, FBC, O], fp8, tag="w2", name=f"w2_{ph}_{fb}")
                    nc.sync.dma_start(w2t[:], w2_d[ph, fb])
                    rw2t = wp2.tile([P, FBC, O], fp8, tag="rw2", name=f"rw2_{ph}_{fb}")
                    nc.sync.dma_start(rw2t[:], rw2_d[ph, fb])

                    # ---- L1 (h, h8, rh8) interleaved with L2 at N-tile grain ----
                    h8p = [hp.tile([P, 2, tph], fp8, tag="h8", name=f"h8_{ph}_{fb}_{j}")
                           for j in range(FBC // 2)]
                    rh8p = [hp.tile([P, 2, tph], fp8, tag="rh8", name=f"rh8_{ph}_{fb}_{j}")
                            for j in range(FBC // 2)]
                    mts_done = 0
                    for (n0, nl) in NT:
                        for fc8 in range(FBC):
                            j, i = fc8 // 2, fc8 % 2
                            gfc = fb * FBC + fc8
                            fsl = slice(fc8 * P, (fc8 + 1) * P)
                            ps = ps1.tile([P, 512], f32, tag="l1",
                                          name=f"l1_{ph}_{fb}_{fc8}_{n0}")
                            mms = []
                            for dj in range(DCH // 2):
                                dsl = slice(2 * dj, 2 * dj + 2)
                                mms.append((w1t[:, dsl, fsl], x8t[:, dsl, n0:n0 + nl]))
                            for dj in range(DCH // 2):
                                dsl = slice(2 * dj, 2 * dj + 2)
                                mms.append((rw1t[:, dsl, fsl], x8t[:, dsl, n0:n0 + nl]))
                            for dj in range(DCH // 2):
                                dsl = slice(2 * dj, 2 * dj + 2)
                                mms.append((w1t[:, dsl, fsl], rx8t[:, dsl, n0:n0 + nl]))
                            for k, (lh, rh_) in enumerate(mms):
                                nc.tensor.matmul(ps[:, :nl], lhsT=lh, rhs=rh_,
                                                 start=(k == 0), stop=(k == len(mms) - 1),
                                                 perf_mode=PM)
                            hb = hbp.tile([P, 512], bf, tag="hb",
                                          name=f"hb_{ph}_{fb}_{fc8}_{n0}")
                            nc.scalar.activation(hb[:, :nl], ps[:, :nl], AF.Relu,
                                                 bias=b1t[:, gfc:gfc + 1], scale=1.0 / S1)
                            nc.scalar.activation(h8p[j][:, i, n0:n0 + nl], ps[:, :nl],
                                                 AF.Relu, bias=b1t[:, gfc:gfc + 1],
                                                 scale=1.0 / S1)
                            nc.gpsimd.tensor_tensor(rh8p[j][:, i, n0:n0 + nl],
                                                    hb[:, :nl], h8p[j][:, i, n0:n0 + nl],
                                                    ALU.subtract)
                        # ---- L2 for token tiles fully covered so far ----
                        for mt in range(mts_done, (n0 + nl) // P):
                            msl = slice(mt * P, (mt + 1) * P)
                            ot = op.tile([P, O], bf, tag="out", name=f"ot_{ph}_{fb}_{mt}")
                            for oh in range(NOH):
                                osl = slice(oh * 512, (oh + 1) * 512)
                                pso = ps2.tile([P, 512], f32, tag="l2",
                                               name=f"l2_{ph}_{fb}_{mt}_{oh}")
                                mms = []
                                for j in range(FBC // 2):
                                    jsl = slice(2 * j, 2 * j + 2)
                                    mms.append((h8p[j][:, :, msl], w2t[:, jsl, osl], PM))
                                for j in range(FBC // 2):
                                    jsl = slice(2 * j, 2 * j + 2)
                                    mms.append((h8p[j][:, :, msl], rw2t[:, jsl, osl], PM))
                                for j in range(FBC // 2):
                                    jsl = slice(2 * j, 2 * j + 2)
                                    mms.append((rh8p[j][:, :, msl], w2t[:, jsl, osl], PM))
                                if fb == NFB - 1 and with_b2:
                                    mms.append((onesbf[:], b2t[:, osl], None))
                                for k, (lh, rh_, pm) in enumerate(mms):
                                    nc.tensor.matmul(pso[:], lhsT=lh, rhs=rh_,
                                                     start=(k == 0),
                                                     stop=(k == len(mms) - 1),
                                                     perf_mode=pm)
                                nc.vector.tensor_scalar(ot[:, osl], pso[:],
                                                        cft[:, mt:mt + 1], None, ALU.mult)
                            nc.sync.dma_start(out_d[ph, fb, mt], ot[:])
                        mts_done = (n0 + nl) // P

    nc.finalize()
    return nc


def _get_nc(key=None):
    if key is None:
        key = _LAST_TPH[0]
    if isinstance(key, int):
        key = (key, True)
    if key not in _CACHED:
        _CACHED[key] = _build(*key)
    return _CACHED[key]


def _q8(a):
    a8 = a.astype(F8)
    r8 = (a - a8.astype(np.float32)).astype(F8)
    return a8, r8


def _prep(x, gate_w, gate_b, sw1, sb1, sw2, sb2, rw1, rb1, rw2, rb2):
    xf = np.ascontiguousarray(np.asarray(x, np.float32).reshape(TOK, D))
    logits = xf @ np.asarray(gate_w, np.float32) + np.asarray(gate_b, np.float32)
    m1 = logits.max(1, keepdims=True)
    ex = np.exp(logits - m1)
    probs = (ex / ex.sum(1, keepdims=True)).astype(np.float32)
    pm = logits + (logits >= m1) * np.float32(-1e30)
    keep = logits >= pm.max(1, keepdims=True)   # top-2 mask (same math as ref)

    allt = np.arange(TOK)
    segs = [(n, allt, np.full(TOK, 0.5, np.float32)) for n in range(NS)]
    for r in range(E):
        idx = np.nonzero(keep[:, r])[0]
        segs.append((NS + r, idx, probs[idx, r]))

    nslots = NCORES * NPH
    tph = P
    while sum((len(sg[1]) + tph - 1) // tph for sg in segs) > nslots:
        tph += P

    slots = []
    for e, idx, cf in segs:
        for s0 in range(0, len(idx), tph):
            slots.append((e, idx[s0:s0 + tph], cf[s0:s0 + tph]))
    while len(slots) < nslots:
        slots.append((0, np.zeros(0, np.int64), np.zeros(0, np.float32)))

    # per-expert quantized weights in device layout
    w1_all = np.concatenate([np.asarray(sw1, np.float32),
                             np.asarray(rw1, np.float32)], 0)
    w2_all = np.concatenate([np.asarray(sw2, np.float32),
                             np.asarray(rw2, np.float32)], 0)
    b1_all = np.concatenate([np.asarray(sb1, np.float32),
                             np.asarray(rb1, np.float32)], 0)
    b2_all = np.concatenate([np.asarray(sb2, np.float32),
                             np.asarray(rb2, np.float32)], 0)
    W1L, RW1L, W2L, RW2L, B1L, B2L = {}, {}, {}, {}, {}, {}
    for e in range(NEXP):
        a8, r8 = _q8(S1 * w1_all[e])                       # [D, F]
        W1L[e] = np.ascontiguousarray(
            a8.reshape(DCH, P, NFB, FBW).transpose(2, 1, 0, 3))
        RW1L[e] = np.ascontiguousarray(
            r8.reshape(DCH, P, NFB, FBW).transpose(2, 1, 0, 3))
        a8, r8 = _q8(S2 * w2_all[e])                       # [F, O]
        W2L[e] = np.ascontiguousarray(
            a8.reshape(NFB, FBC, P, O).transpose(0, 2, 1, 3))
        RW2L[e] = np.ascontiguousarray(
            r8.reshape(NFB, FBC, P, O).transpose(0, 2, 1, 3))
        B1L[e] = np.ascontiguousarray(b1_all[e].reshape(FCH, P).T).astype(np.float32)
        B2L[e] = (S2 * b2_all[e]).reshape(1, O).astype(BF16)

    x8_full, rx8_full = _q8(xf)                            # [TOK, D] fp8

    def slot_x(arr, idx):
        g = np.zeros((tph, D), F8)
        g[:len(idx)] = arr[idx]
        return np.ascontiguousarray(g.T.reshape(DCH, P, tph).transpose(1, 0, 2))

    MT = tph // P
    in_maps = []
    for c in range(NCORES):
        csl = slots[c * NPH:(c + 1) * NPH]
        cfarr = np.zeros((NPH, P, MT), np.float32)
        for s, (e, idx, cf) in enumerate(csl):
            v = np.zeros(tph, np.float32)
            v[:len(idx)] = cf / S2
            cfarr[s] = v.reshape(MT, P).T
        in_maps.append({
            "x8": np.stack([slot_x(x8_full, sg[1]) for sg in csl]),
            "rx8": np.stack([slot_x(rx8_full, sg[1]) for sg in csl]),
            "w1": np.stack([W1L[sg[0]] for sg in csl]),
            "rw1": np.stack([RW1L[sg[0]] for sg in csl]),
            "w2": np.stack([W2L[sg[0]] for sg in csl]),
            "rw2": np.stack([RW2L[sg[0]] for sg in csl]),
            "b1": np.stack([B1L[sg[0]] for sg in csl]),
            "b2": np.stack([B2L[sg[0]] for sg in csl]),
            "cf": cfarr,
        })
    return in_maps, slots, tph


def kernel(**inputs) -> np.ndarray:
    in_maps, slots, tph = _prep(**inputs)
    with_b2 = bool(np.any(np.asarray(inputs["sb2"])) or np.any(np.asarray(inputs["rb2"])))
    _LAST_TPH[0] = (tph, with_b2)
    nc = _get_nc((tph, with_b2))
    res = run_bass_kernel_spmd(nc, in_maps, list(range(NCORES)))
    out = np.zeros((TOK, O), np.float32)
    for c in range(NCORES):
        og = np.asarray(res.results[c]["out"], np.float32).sum(1)  # [NPH, MT, P, O]
        for s in range(NPH):
            e, idx, cf = slots[c * NPH + s]
            n = len(idx)
            if n:
                out[idx] += og[s].reshape(tph, O)[:n]
    return out.reshape(B, S, O).astype(np.float32)


# revision 12
# speedup vs baseline: 1.0537x; 1.0537x over previous
"""Trainium2 Bass kernel for the MoE layer (4 routed top-2 + 2 shared experts).

Strategy (v2): exact expert-parallel routing on the host + fp8 DoubleRow
matmuls with 3-term residual compensation on the device.

  - Host computes the fp32 gating softmax/top-2 and builds a global work
    list of (expert, token) pairs: 2 shared experts x all 8192 tokens +
    top-2 routed picks (16384 pairs) = 32768 pairs total. The list is cut
    into 24 slots of TPH=1408 tokens (one expert per slot, zero-coef
    padding) and distributed 3 slots (phases) per core: 4224 pairs/core.
  - Each phase streams one expert's weights and applies the expert MLP to
    its tokens; per-token fp32 coefficients (0.5 for shared, softmax prob
    for routed) are applied at the output drain; host scatter-adds slot
    outputs back into the full [8192, 1024] output.
  - All matmuls run as fp8e4m3 DoubleRow (0.5 cycles/row, 256-deep
    contraction) with 3-term residual compensation, which costs 0.75x of a
    bf16 schedule but keeps ~bf16 accuracy:
        x@W ~= x8@W8 + rx8@W8 + x8@RW8
    where x8=fp8(x), rx8=fp8(x-x8), W8=fp8(S*W), RW8=fp8(S*W-W8). The
    residuals need no extra scaling (fp8 subnormals cover their range);
    the 1/S weight scale folds into the activation/drain scales.
  - L1 produces h in bf16 (ACT relu) plus h8=fp8 (second ACT relu) and
    rh8=hb-h8 (Pool-engine subtract); L2 consumes (h8, rh8) the same way.
  - L2 is F-blocked (4 blocks of 1024); each block's partial output drains
    through DVE (coef/S2 per-token scale) to bf16 and is summed on host.
"""

import sys

sys.path.insert(0, '/opt/trn_rl_repo')

import numpy as np
import ml_dtypes

import concourse.bass as bass
import concourse.mybir as mybir
import concourse.tile as tile
from concourse import bacc
from concourse.bass_utils import run_bass_kernel_spmd

F8 = ml_dtypes.float8_e4m3
BF16 = ml_dtypes.bfloat16

NCORES = 8
B, S, D, F, O = 4, 2048, 1024, 4096, 1024
E, NS = 4, 2
NEXP = NS + E
TOK = B * S
P = 128
DCH = D // P          # 8 contraction chunks in L1
FCH = F // P          # 32
NFB = 4               # L2 F-blocks
FBC = FCH // NFB      # 8 f-chunks per block
FBW = FBC * P         # 1024 f-cols per block
NPH = 3               # phases (slots) per core
S1, S2 = 32.0, 64.0   # weight pre-scales (powers of 2)

_CACHED = {}
_LAST_TPH = [(1408, False)]


def _ntiles(tph):
    out, n0 = [], 0
    while n0 < tph:
        nl = min(512, tph - n0)
        out.append((n0, nl))
        n0 += nl
    return out


def _build(tph, with_b2=True):
    f32 = mybir.dt.float32
    bf = mybir.dt.bfloat16
    fp8 = mybir.dt.float8e4
    AF = mybir.ActivationFunctionType
    ALU = mybir.AluOpType
    PM = mybir.MatmulPerfMode.DoubleRow
    MT = tph // P
    NT = _ntiles(tph)
    NOH = O // 512

    nc = bacc.Bacc("TRN2", target_bir_lowering=False, debug=False)

    x8_d = nc.dram_tensor("x8", [NPH, P, DCH, tph], fp8, kind="ExternalInput")
    rx8_d = nc.dram_tensor("rx8", [NPH, P, DCH, tph], fp8, kind="ExternalInput")
    w1_d = nc.dram_tensor("w1", [NPH, NFB, P, DCH, FBW], fp8, kind="ExternalInput")
    rw1_d = nc.dram_tensor("rw1", [NPH, NFB, P, DCH, FBW], fp8, kind="ExternalInput")
    w2_d = nc.dram_tensor("w2", [NPH, NFB, P, FBC, O], fp8, kind="ExternalInput")
    rw2_d = nc.dram_tensor("rw2", [NPH, NFB, P, FBC, O], fp8, kind="ExternalInput")
    b1_d = nc.dram_tensor("b1", [NPH, P, FCH], f32, kind="ExternalInput")
    b2_d = nc.dram_tensor("b2", [NPH, 1, O], bf, kind="ExternalInput")
    cf_d = nc.dram_tensor("cf", [NPH, P, MT], f32, kind="ExternalInput")
    out_d = nc.dram_tensor("out", [NPH, NFB, MT, P, O], bf, kind="ExternalOutput")

    with tile.TileContext(nc) as tc:
        with (
            tc.tile_pool(name="xp", bufs=2) as xp,
            tc.tile_pool(name="wp1", bufs=2) as wp1,
            tc.tile_pool(name="wp2", bufs=2) as wp2,
            tc.tile_pool(name="hp", bufs=8) as hp,
            tc.tile_pool(name="hbp", bufs=4) as hbp,
            tc.tile_pool(name="op", bufs=8) as op,
            tc.tile_pool(name="cp", bufs=2) as cp,
            tc.tile_pool(name="consts", bufs=1) as consts,
            tc.tile_pool(name="ps1", bufs=4, space="PSUM") as ps1,
            tc.tile_pool(name="ps2", bufs=4, space="PSUM") as ps2,
        ):
            onesbf = consts.tile([1, P], bf, tag="ones", name="ones")
            nc.vector.memset(onesbf[:], 1.0)

            # DMA queue split: x8/out on SP (nc.sync), weights/consts on the
            # Activation HWDGE queue (nc.scalar) — halves descriptor-gen
            # serialization and gets first-needed data in flight sooner.
            th = (tph // 2) // P * P
            for ph in range(NPH):
                x8t = xp.tile([P, DCH, tph], fp8, tag="x8", name=f"x8_{ph}")
                nc.sync.dma_start(x8t[:, :, 0:th], x8_d[ph, :, :, 0:th])
                rx8t = xp.tile([P, DCH, tph], fp8, tag="rx8", name=f"rx8_{ph}")
                nc.scalar.dma_start(rx8t[:, :, 0:th], rx8_d[ph, :, :, 0:th])
                nc.sync.dma_start(x8t[:, :, th:tph], x8_d[ph, :, :, th:tph])
                nc.scalar.dma_start(rx8t[:, :, th:tph], rx8_d[ph, :, :, th:tph])
                b1t = cp.tile([P, FCH], f32, tag="b1", name=f"b1_{ph}")
                nc.sync.dma_start(b1t[:], b1_d[ph])
                b2t = cp.tile([1, O], bf, tag="b2", name=f"b2_{ph}")
                nc.sync.dma_start(b2t[:], b2_d[ph])
                cft = cp.tile([P, MT], f32, tag="cf", name=f"cf_{ph}")
                nc.sync.dma_start(cft[:], cf_d[ph])

                for fb in range(NFB):
                    fh = FBW // 2
                    w1t = wp1.tile([P, DCH, FBW], fp8, tag="w1", name=f"w1_{ph}_{fb}")
                    nc.sync.dma_start(w1t[:, :, 0:fh], w1_d[ph, fb, :, :, 0:fh])
                    rw1t = wp1.tile([P, DCH, FBW], fp8, tag="rw1", name=f"rw1_{ph}_{fb}")
                    nc.scalar.dma_start(rw1t[:, :, 0:fh], rw1_d[ph, fb, :, :, 0:fh])
                    nc.sync.dma_start(w1t[:, :, fh:FBW], w1_d[ph, fb, :, :, fh:FBW])
                    nc.scalar.dma_start(rw1t[:, :, fh:FBW], rw1_d[ph, fb, :, :, fh:FBW])
                    w2t = wp2.tile([P, FBC, O], fp8, tag="w2", name=f"w2_{ph}_{fb}")
                    nc.scalar.dma_start(w2t[:], w2_d[ph, fb])
                    rw2t = wp2.tile([P, FBC, O], fp8, tag="rw2", name=f"rw2_{ph}_{fb}")
                    nc.scalar.dma_start(rw2t[:], rw2_d[ph, fb])

                    # ---- L1 (h, h8, rh8) interleaved with L2 at N-tile grain ----
                    h8p = [hp.tile([P, 2, tph], fp8, tag="h8", name=f"h8_{ph}_{fb}_{j}")
                           for j in range(FBC // 2)]
                    rh8p = [hp.tile([P, 2, tph], fp8, tag="rh8", name=f"rh8_{ph}_{fb}_{j}")
                            for j in range(FBC // 2)]
                    mts_done = 0
                    for (n0, nl) in NT:
                        for fc8 in range(FBC):
                            j, i = fc8 // 2, fc8 % 2
                            gfc = fb * FBC + fc8
                            fsl = slice(fc8 * P, (fc8 + 1) * P)
                            ps = ps1.tile([P, 512], f32, tag="l1",
                                          name=f"l1_{ph}_{fb}_{fc8}_{n0}")
                            mms = []
                            for dj in range(DCH // 2):
                                dsl = slice(2 * dj, 2 * dj + 2)
                                mms.append((w1t[:, dsl, fsl], x8t[:, dsl, n0:n0 + nl]))
                            for dj in range(DCH // 2):
                                dsl = slice(2 * dj, 2 * dj + 2)
                                mms.append((rw1t[:, dsl, fsl], x8t[:, dsl, n0:n0 + nl]))
                            for dj in range(DCH // 2):
                                dsl = slice(2 * dj, 2 * dj + 2)
                                mms.append((w1t[:, dsl, fsl], rx8t[:, dsl, n0:n0 + nl]))
                            for k, (lh, rh_) in enumerate(mms):
                                nc.tensor.matmul(ps[:, :nl], lhsT=lh, rhs=rh_,
                                                 start=(k == 0), stop=(k == len(mms) - 1),
                                                 perf_mode=PM)
                            hb = hbp.tile([P, 512], bf, tag="hb",
                                          name=f"hb_{ph}_{fb}_{fc8}_{n0}")
                            nc.scalar.activation(hb[:, :nl], ps[:, :nl], AF.Relu,
                                                 bias=b1t[:, gfc:gfc + 1], scale=1.0 / S1)
                            nc.vector.tensor_copy(h8p[j][:, i, n0:n0 + nl], hb[:, :nl])
                            nc.gpsimd.tensor_tensor(rh8p[j][:, i, n0:n0 + nl],
                                                    hb[:, :nl], h8p[j][:, i, n0:n0 + nl],
                                                    ALU.subtract)
                        # ---- L2 for token tiles fully covered so far ----
                        for mt in range(mts_done, (n0 + nl) // P):
                            msl = slice(mt * P, (mt + 1) * P)
                            ot = op.tile([P, O], bf, tag="out", name=f"ot_{ph}_{fb}_{mt}")
                            for oh in range(NOH):
                                osl = slice(oh * 512, (oh + 1) * 512)
                                pso = ps2.tile([P, 512], f32, tag="l2",
                                               name=f"l2_{ph}_{fb}_{mt}_{oh}")
                                mms = []
                                for j in range(FBC // 2):
                                    jsl = slice(2 * j, 2 * j + 2)
                                    mms.append((h8p[j][:, :, msl], w2t[:, jsl, osl], PM))
                                for j in range(FBC // 2):
                                    jsl = slice(2 * j, 2 * j + 2)
                                    mms.append((h8p[j][:, :, msl], rw2t[:, jsl, osl], PM))
                                for j in range(FBC // 2):
                                    jsl = slice(2 * j, 2 * j + 2)
                                    mms.append((rh8p[j][:, :, msl], w2t[:, jsl, osl], PM))
                                if fb == NFB - 1 and with_b2:
                                    mms.append((onesbf[:], b2t[:, osl], None))
                                for k, (lh, rh_, pm) in enumerate(mms):
                                    nc.tensor.matmul(pso[:], lhsT=lh, rhs=rh_,
                                                     start=(k == 0),
                                                     stop=(k == len(mms) - 1),
                                                     perf_mode=pm)
                                nc.vector.tensor_scalar(ot[:, osl], pso[:],
                                                        cft[:, mt:mt + 1], None, ALU.mult)
                            nc.sync.dma_start(out_d[ph, fb, mt], ot[:])
                        mts_done = (n0 + nl) // P

    nc.finalize()
    return nc


def _get_nc(key=None):
    if key is None:
        key = _LAST_TPH[0]
    if isinstance(key, int):
        key = (key, True)
    if key not in _CACHED:
        _CACHED[key] = _build(*key)
    return _CACHED[key]


def _q8(a):
    a8 = a.astype(F8)
    r8 = (a - a8.astype(np.float32)).astype(F8)
    return a8, r8


def _prep(x, gate_w, gate_b, sw1, sb1, sw2, sb2, rw1, rb1, rw2, rb2):
    xf = np.ascontiguousarray(np.asarray(x, np.float32).reshape(TOK, D))
    logits = xf @ np.asarray(gate_w, np.float32) + np.asarray(gate_b, np.float32)
    m1 = logits.max(1, keepdims=True)
    ex = np.exp(logits - m1)
    probs = (ex / ex.sum(1, keepdims=True)).astype(np.float32)
    pm = logits + (logits >= m1) * np.float32(-1e30)
    keep = logits >= pm.max(1, keepdims=True)   # top-2 mask (same math as ref)

    allt = np.arange(TOK)
    segs = [(n, allt, np.full(TOK, 0.5, np.float32)) for n in range(NS)]
    for r in range(E):
        idx = np.nonzero(keep[:, r])[0]
        segs.append((NS + r, idx, probs[idx, r]))

    nslots = NCORES * NPH
    tph = P
    while sum((len(sg[1]) + tph - 1) // tph for sg in segs) > nslots:
        tph += P

    slots = []
    for e, idx, cf in segs:
        for s0 in range(0, len(idx), tph):
            slots.append((e, idx[s0:s0 + tph], cf[s0:s0 + tph]))
    while len(slots) < nslots:
        slots.append((0, np.zeros(0, np.int64), np.zeros(0, np.float32)))

    # per-expert quantized weights in device layout
    w1_all = np.concatenate([np.asarray(sw1, np.float32),
                             np.asarray(rw1, np.float32)], 0)
    w2_all = np.concatenate([np.asarray(sw2, np.float32),
                             np.asarray(rw2, np.float32)], 0)
    b1_all = np.concatenate([np.asarray(sb1, np.float32),
                             np.asarray(rb1, np.float32)], 0)
    b2_all = np.concatenate([np.asarray(sb2, np.float32),
                             np.asarray(rb2, np.float32)], 0)
    W1L, RW1L, W2L, RW2L, B1L, B2L = {}, {}, {}, {}, {}, {}
    for e in range(NEXP):
        a8, r8 = _q8(S1 * w1_all[e])                       # [D, F]
        W1L[e] = np.ascontiguousarray(
            a8.reshape(DCH, P, NFB, FBW).transpose(2, 1, 0, 3))
        RW1L[e] = np.ascontiguousarray(
            r8.reshape(DCH, P, NFB, FBW).transpose(2, 1, 0, 3))
        a8, r8 = _q8(S2 * w2_all[e])                       # [F, O]
        W2L[e] = np.ascontiguousarray(
            a8.reshape(NFB, FBC, P, O).transpose(0, 2, 1, 3))
        RW2L[e] = np.ascontiguousarray(
            r8.reshape(NFB, FBC, P, O).transpose(0, 2, 1, 3))
        B1L[e] = np.ascontiguousarray(b1_all[e].reshape(FCH, P).T).astype(np.float32)
        B2L[e] = (S2 * b2_all[e]).reshape(1, O).astype(BF16)

    x8_full, rx8_full = _q8(xf)                            # [TOK, D] fp8

    def slot_x(arr, idx):
        g = np.zeros((tph, D), F8)
        g[:len(idx)] = arr[idx]
        return np.ascontiguousarray(g.T.reshape(DCH, P, tph).transpose(1, 0, 2))

    MT = tph // P
    in_maps = []
    for c in range(NCORES):
        csl = slots[c * NPH:(c + 1) * NPH]
        cfarr = np.zeros((NPH, P, MT), np.float32)
        for s, (e, idx, cf) in enumerate(csl):
            v = np.zeros(tph, np.float32)
            v[:len(idx)] = cf / S2
            cfarr[s] = v.reshape(MT, P).T
        in_maps.append({
            "x8": np.stack([slot_x(x8_full, sg[1]) for sg in csl]),
            "rx8": np.stack([slot_x(rx8_full, sg[1]) for sg in csl]),
            "w1": np.stack([W1L[sg[0]] for sg in csl]),
            "rw1": np.stack([RW1L[sg[0]] for sg in csl]),
            "w2": np.stack([W2L[sg[0]] for sg in csl]),
            "rw2": np.stack([RW2L[sg[0]] for sg in csl]),
            "b1": np.stack([B1L[sg[0]] for sg in csl]),
            "b2": np.stack([B2L[sg[0]] for sg in csl]),
            "cf": cfarr,
        })
    return in_maps, slots, tph


def kernel(**inputs) -> np.ndarray:
    in_maps, slots, tph = _prep(**inputs)
    with_b2 = bool(np.any(np.asarray(inputs["sb2"])) or np.any(np.asarray(inputs["rb2"])))
    _LAST_TPH[0] = (tph, with_b2)
    nc = _get_nc((tph, with_b2))
    res = run_bass_kernel_spmd(nc, in_maps, list(range(NCORES)))
    out = np.zeros((TOK, O), np.float32)
    for c in range(NCORES):
        og = np.asarray(res.results[c]["out"], np.float32).sum(1)  # [NPH, MT, P, O]
        for s in range(NPH):
            e, idx, cf = slots[c * NPH + s]
            n = len(idx)
            if n:
                out[idx] += og[s].reshape(tph, O)[:n]
    return out.reshape(B, S, O).astype(np.float32)


# revision 19
# speedup vs baseline: 1.1531x; 1.0943x over previous
"""Trainium2 Bass kernel for the MoE layer (4 routed top-2 + 2 shared experts).

Strategy (v2): exact expert-parallel routing on the host + fp8 DoubleRow
matmuls with 3-term residual compensation on the device.

  - Host computes the fp32 gating softmax/top-2 and builds a global work
    list of (expert, token) pairs: 2 shared experts x all 8192 tokens +
    top-2 routed picks (16384 pairs) = 32768 pairs total. The list is cut
    into 24 slots of TPH=1408 tokens (one expert per slot, zero-coef
    padding) and distributed 3 slots (phases) per core: 4224 pairs/core.
  - Each phase streams one expert's weights and applies the expert MLP to
    its tokens; per-token fp32 coefficients (0.5 for shared, softmax prob
    for routed) are applied at the output drain; host scatter-adds slot
    outputs back into the full [8192, 1024] output.
  - All matmuls run as fp8e4m3 DoubleRow (0.5 cycles/row, 256-deep
    contraction) with 3-term residual compensation, which costs 0.75x of a
    bf16 schedule but keeps ~bf16 accuracy:
        x@W ~= x8@W8 + rx8@W8 + x8@RW8
    where x8=fp8(x), rx8=fp8(x-x8), W8=fp8(S*W), RW8=fp8(S*W-W8). The
    residuals need no extra scaling (fp8 subnormals cover their range);
    the 1/S weight scale folds into the activation/drain scales.
  - L1 produces h in bf16 (ACT relu) plus h8=fp8 (second ACT relu) and
    rh8=hb-h8 (Pool-engine subtract); L2 consumes (h8, rh8) the same way.
  - L2 is F-blocked (4 blocks of 1024); each block's partial output drains
    through DVE (coef/S2 per-token scale) to bf16 and is summed on host.
"""

import sys

sys.path.insert(0, '/opt/trn_rl_repo')

import numpy as np
import ml_dtypes

import concourse.bass as bass
import concourse.mybir as mybir
import concourse.tile as tile
from concourse import bacc
from concourse.bass_utils import run_bass_kernel_spmd

F8 = ml_dtypes.float8_e4m3
BF16 = ml_dtypes.bfloat16

NCORES = 8
B, S, D, F, O = 4, 2048, 1024, 4096, 1024
E, NS = 4, 2
NEXP = NS + E
TOK = B * S
P = 128
DCH = D // P          # 8 contraction chunks in L1
FCH = F // P          # 32
NFB = 4               # L2 F-blocks
FBC = FCH // NFB      # 8 f-chunks per block
FBW = FBC * P         # 1024 f-cols per block
NPH = 3               # phases (slots) per core
S1, S2 = 32.0, 64.0   # weight pre-scales (powers of 2)

_CACHED = {}
_LAST_TPH = [(1408, False)]


def _ntiles(tph):
    out, n0 = [], 0
    while n0 < tph:
        nl = min(512, tph - n0)
        out.append((n0, nl))
        n0 += nl
    return out


def _build(tph, with_b2=True, nc1=4, nc2=4):
    # nc1/nc2: how many of the 4 contraction pair-groups carry the
    # input-residual compensation term in L1/L2 (4 = full compensation).
    f32 = mybir.dt.float32
    bf = mybir.dt.bfloat16
    fp8 = mybir.dt.float8e4
    AF = mybir.ActivationFunctionType
    ALU = mybir.AluOpType
    PM = mybir.MatmulPerfMode.DoubleRow
    MT = tph // P
    NT = _ntiles(tph)
    NOH = O // 512

    nc = bacc.Bacc("TRN2", target_bir_lowering=False, debug=False)

    x8_d = nc.dram_tensor("x8", [NPH, P, DCH, tph], fp8, kind="ExternalInput")
    rx8_d = nc.dram_tensor("rx8", [NPH, P, DCH, tph], fp8, kind="ExternalInput")
    w1_d = nc.dram_tensor("w1", [NPH, NFB, P, DCH, FBW], fp8, kind="ExternalInput")
    rw1_d = nc.dram_tensor("rw1", [NPH, NFB, P, DCH, FBW], fp8, kind="ExternalInput")
    w2_d = nc.dram_tensor("w2", [NPH, NFB, P, FBC, O], fp8, kind="ExternalInput")
    rw2_d = nc.dram_tensor("rw2", [NPH, NFB, P, FBC, O], fp8, kind="ExternalInput")
    b1_d = nc.dram_tensor("b1", [NPH, P, FCH], f32, kind="ExternalInput")
    b2_d = nc.dram_tensor("b2", [NPH, 1, O], bf, kind="ExternalInput")
    cf_d = nc.dram_tensor("cf", [NPH, P, MT], f32, kind="ExternalInput")
    out_d = nc.dram_tensor("out", [NPH, NFB, MT, P, O], bf, kind="ExternalOutput")

    with tile.TileContext(nc) as tc:
        with (
            tc.tile_pool(name="xp", bufs=2) as xp,
            tc.tile_pool(name="wp1", bufs=2) as wp1,
            tc.tile_pool(name="wp2", bufs=2) as wp2,
            tc.tile_pool(name="hp", bufs=8) as hp,
            tc.tile_pool(name="hbp", bufs=4) as hbp,
            tc.tile_pool(name="op", bufs=8) as op,
            tc.tile_pool(name="cp", bufs=2) as cp,
            tc.tile_pool(name="consts", bufs=1) as consts,
            tc.tile_pool(name="ps1", bufs=4, space="PSUM") as ps1,
            tc.tile_pool(name="ps2", bufs=4, space="PSUM") as ps2,
        ):
            onesbf = consts.tile([1, P], bf, tag="ones", name="ones")
            nc.vector.memset(onesbf[:], 1.0)

            # DMA queue split: x8/out on SP (nc.sync), weights/consts on the
            # Activation HWDGE queue (nc.scalar) — halves descriptor-gen
            # serialization and gets first-needed data in flight sooner.
            th = (tph // 2) // P * P
            for ph in range(NPH):
                dl = 2 * nc1
                x8t = xp.tile([P, DCH, tph], fp8, tag="x8", name=f"x8_{ph}")
                nc.sync.dma_start(x8t[:, :, 0:th], x8_d[ph, :, :, 0:th])
                rx8t = xp.tile([P, DCH, tph], fp8, tag="rx8", name=f"rx8_{ph}")
                nc.scalar.dma_start(rx8t[:, 0:dl, 0:th], rx8_d[ph, :, 0:dl, 0:th])
                nc.sync.dma_start(x8t[:, :, th:tph], x8_d[ph, :, :, th:tph])
                nc.scalar.dma_start(rx8t[:, 0:dl, th:tph], rx8_d[ph, :, 0:dl, th:tph])
                b1t = cp.tile([P, FCH], f32, tag="b1", name=f"b1_{ph}")
                nc.sync.dma_start(b1t[:], b1_d[ph])
                b2t = cp.tile([1, O], bf, tag="b2", name=f"b2_{ph}")
                nc.sync.dma_start(b2t[:], b2_d[ph])
                cft = cp.tile([P, MT], f32, tag="cf", name=f"cf_{ph}")
                nc.sync.dma_start(cft[:], cf_d[ph])

                for fb in range(NFB):
                    fh = FBW // 2
                    w1t = wp1.tile([P, DCH, FBW], fp8, tag="w1", name=f"w1_{ph}_{fb}")
                    nc.sync.dma_start(w1t[:, :, 0:fh], w1_d[ph, fb, :, :, 0:fh])
                    rw1t = wp1.tile([P, DCH, FBW], fp8, tag="rw1", name=f"rw1_{ph}_{fb}")
                    nc.scalar.dma_start(rw1t[:, :, 0:fh], rw1_d[ph, fb, :, :, 0:fh])
                    nc.sync.dma_start(w1t[:, :, fh:FBW], w1_d[ph, fb, :, :, fh:FBW])
                    nc.scalar.dma_start(rw1t[:, :, fh:FBW], rw1_d[ph, fb, :, :, fh:FBW])
                    w2t = wp2.tile([P, FBC, O], fp8, tag="w2", name=f"w2_{ph}_{fb}")
                    nc.scalar.dma_start(w2t[:], w2_d[ph, fb])
                    rw2t = wp2.tile([P, FBC, O], fp8, tag="rw2", name=f"rw2_{ph}_{fb}")
                    nc.scalar.dma_start(rw2t[:], rw2_d[ph, fb])

                    # ---- L1 (h, h8, rh8) interleaved with L2 at N-tile grain ----
                    h8p = [hp.tile([P, 2, tph], fp8, tag="h8", name=f"h8_{ph}_{fb}_{j}")
                           for j in range(FBC // 2)]
                    rh8p = [hp.tile([P, 2, tph], fp8, tag="rh8", name=f"rh8_{ph}_{fb}_{j}")
                            for j in range(nc2)]
                    mts_done = 0
                    for (n0, nl) in NT:
                        for fc8 in range(FBC):
                            j, i = fc8 // 2, fc8 % 2
                            gfc = fb * FBC + fc8
                            fsl = slice(fc8 * P, (fc8 + 1) * P)
                            ps = ps1.tile([P, 512], f32, tag="l1",
                                          name=f"l1_{ph}_{fb}_{fc8}_{n0}")
                            mms = []
                            for dj in range(DCH // 2):
                                dsl = slice(2 * dj, 2 * dj + 2)
                                mms.append((w1t[:, dsl, fsl], x8t[:, dsl, n0:n0 + nl]))
                            for dj in range(DCH // 2):
                                dsl = slice(2 * dj, 2 * dj + 2)
                                mms.append((rw1t[:, dsl, fsl], x8t[:, dsl, n0:n0 + nl]))
                            for dj in range(nc1):
                                dsl = slice(2 * dj, 2 * dj + 2)
                                mms.append((w1t[:, dsl, fsl], rx8t[:, dsl, n0:n0 + nl]))
                            for k, (lh, rh_) in enumerate(mms):
                                nc.tensor.matmul(ps[:, :nl], lhsT=lh, rhs=rh_,
                                                 start=(k == 0), stop=(k == len(mms) - 1),
                                                 perf_mode=PM)
                            hb = hbp.tile([P, 512], bf, tag="hb",
                                          name=f"hb_{ph}_{fb}_{fc8}_{n0}")
                            nc.scalar.activation(hb[:, :nl], ps[:, :nl], AF.Relu,
                                                 bias=b1t[:, gfc:gfc + 1], scale=1.0 / S1)
                            nc.vector.tensor_copy(h8p[j][:, i, n0:n0 + nl], hb[:, :nl])
                            if j < nc2:
                                nc.gpsimd.tensor_tensor(rh8p[j][:, i, n0:n0 + nl],
                                                        hb[:, :nl],
                                                        h8p[j][:, i, n0:n0 + nl],
                                                        ALU.subtract)
                        # ---- L2 for token tiles fully covered so far ----
                        for mt in range(mts_done, (n0 + nl) // P):
                            msl = slice(mt * P, (mt + 1) * P)
                            ot = op.tile([P, O], bf, tag="out", name=f"ot_{ph}_{fb}_{mt}")
                            for oh in range(NOH):
                                osl = slice(oh * 512, (oh + 1) * 512)
                                pso = ps2.tile([P, 512], f32, tag="l2",
                                               name=f"l2_{ph}_{fb}_{mt}_{oh}")
                                mms = []
                                for j in range(FBC // 2):
                                    jsl = slice(2 * j, 2 * j + 2)
                                    mms.append((h8p[j][:, :, msl], w2t[:, jsl, osl], PM))
                                for j in range(FBC // 2):
                                    jsl = slice(2 * j, 2 * j + 2)
                                    mms.append((h8p[j][:, :, msl], rw2t[:, jsl, osl], PM))
                                for j in range(nc2):
                                    jsl = slice(2 * j, 2 * j + 2)
                                    mms.append((rh8p[j][:, :, msl], w2t[:, jsl, osl], PM))
                                if fb == NFB - 1 and with_b2:
                                    mms.append((onesbf[:], b2t[:, osl], None))
                                for k, (lh, rh_, pm) in enumerate(mms):
                                    nc.tensor.matmul(pso[:], lhsT=lh, rhs=rh_,
                                                     start=(k == 0),
                                                     stop=(k == len(mms) - 1),
                                                     perf_mode=pm)
                                nc.vector.tensor_scalar(ot[:, osl], pso[:],
                                                        cft[:, mt:mt + 1], None, ALU.mult)
                            nc.sync.dma_start(out_d[ph, fb, mt], ot[:])
                        mts_done = (n0 + nl) // P

    nc.finalize()
    return nc


NC1, NC2 = 3, 3


def _get_nc(key=None):
    if key is None:
        key = _LAST_TPH[0]
    if isinstance(key, int):
        key = (key, True)
    if key not in _CACHED:
        _CACHED[key] = _build(key[0], key[1], NC1, NC2)
    return _CACHED[key]


def _q8(a):
    a8 = a.astype(F8)
    r8 = (a - a8.astype(np.float32)).astype(F8)
    return a8, r8


def _prep(x, gate_w, gate_b, sw1, sb1, sw2, sb2, rw1, rb1, rw2, rb2):
    xf = np.ascontiguousarray(np.asarray(x, np.float32).reshape(TOK, D))
    logits = xf @ np.asarray(gate_w, np.float32) + np.asarray(gate_b, np.float32)
    m1 = logits.max(1, keepdims=True)
    ex = np.exp(logits - m1)
    probs = (ex / ex.sum(1, keepdims=True)).astype(np.float32)
    pm = logits + (logits >= m1) * np.float32(-1e30)
    keep = logits >= pm.max(1, keepdims=True)   # top-2 mask (same math as ref)

    allt = np.arange(TOK)
    segs = [(n, allt, np.full(TOK, 0.5, np.float32)) for n in range(NS)]
    for r in range(E):
        idx = np.nonzero(keep[:, r])[0]
        segs.append((NS + r, idx, probs[idx, r]))

    nslots = NCORES * NPH
    tph = P
    while sum((len(sg[1]) + tph - 1) // tph for sg in segs) > nslots:
        tph += P

    slots = []
    for e, idx, cf in segs:
        for s0 in range(0, len(idx), tph):
            slots.append((e, idx[s0:s0 + tph], cf[s0:s0 + tph]))
    while len(slots) < nslots:
        slots.append((0, np.zeros(0, np.int64), np.zeros(0, np.float32)))

    # per-expert quantized weights in device layout
    w1_all = np.concatenate([np.asarray(sw1, np.float32),
                             np.asarray(rw1, np.float32)], 0)
    w2_all = np.concatenate([np.asarray(sw2, np.float32),
                             np.asarray(rw2, np.float32)], 0)
    b1_all = np.concatenate([np.asarray(sb1, np.float32),
                             np.asarray(rb1, np.float32)], 0)
    b2_all = np.concatenate([np.asarray(sb2, np.float32),
                             np.asarray(rb2, np.float32)], 0)
    W1L, RW1L, W2L, RW2L, B1L, B2L = {}, {}, {}, {}, {}, {}
    for e in range(NEXP):
        a8, r8 = _q8(S1 * w1_all[e])                       # [D, F]
        W1L[e] = np.ascontiguousarray(
            a8.reshape(DCH, P, NFB, FBW).transpose(2, 1, 0, 3))
        RW1L[e] = np.ascontiguousarray(
            r8.reshape(DCH, P, NFB, FBW).transpose(2, 1, 0, 3))
        a8, r8 = _q8(S2 * w2_all[e])                       # [F, O]
        W2L[e] = np.ascontiguousarray(
            a8.reshape(NFB, FBC, P, O).transpose(0, 2, 1, 3))
        RW2L[e] = np.ascontiguousarray(
            r8.reshape(NFB, FBC, P, O).transpose(0, 2, 1, 3))
        B1L[e] = np.ascontiguousarray(b1_all[e].reshape(FCH, P).T).astype(np.float32)
        B2L[e] = (S2 * b2_all[e]).reshape(1, O).astype(BF16)

    x8_full, rx8_full = _q8(xf)                            # [TOK, D] fp8

    def slot_x(arr, idx):
        g = np.zeros((tph, D), F8)
        g[:len(idx)] = arr[idx]
        return np.ascontiguousarray(g.T.reshape(DCH, P, tph).transpose(1, 0, 2))

    MT = tph // P
    in_maps = []
    for c in range(NCORES):
        csl = slots[c * NPH:(c + 1) * NPH]
        cfarr = np.zeros((NPH, P, MT), np.float32)
        for s, (e, idx, cf) in enumerate(csl):
            v = np.zeros(tph, np.float32)
            v[:len(idx)] = cf / S2
            cfarr[s] = v.reshape(MT, P).T
        in_maps.append({
            "x8": np.stack([slot_x(x8_full, sg[1]) for sg in csl]),
            "rx8": np.stack([slot_x(rx8_full, sg[1]) for sg in csl]),
            "w1": np.stack([W1L[sg[0]] for sg in csl]),
            "rw1": np.stack([RW1L[sg[0]] for sg in csl]),
            "w2": np.stack([W2L[sg[0]] for sg in csl]),
            "rw2": np.stack([RW2L[sg[0]] for sg in csl]),
            "b1": np.stack([B1L[sg[0]] for sg in csl]),
            "b2": np.stack([B2L[sg[0]] for sg in csl]),
            "cf": cfarr,
        })
    return in_maps, slots, tph


def kernel(**inputs) -> np.ndarray:
    in_maps, slots, tph = _prep(**inputs)
    with_b2 = bool(np.any(np.asarray(inputs["sb2"])) or np.any(np.asarray(inputs["rb2"])))
    _LAST_TPH[0] = (tph, with_b2)
    nc = _get_nc((tph, with_b2))
    res = run_bass_kernel_spmd(nc, in_maps, list(range(NCORES)))
    out = np.zeros((TOK, O), np.float32)
    for c in range(NCORES):
        og = np.asarray(res.results[c]["out"], np.float32).sum(1)  # [NPH, MT, P, O]
        for s in range(NPH):
            e, idx, cf = slots[c * NPH + s]
            n = len(idx)
            if n:
                out[idx] += og[s].reshape(tph, O)[:n]
    return out.reshape(B, S, O).astype(np.float32)
